# revision 14
# baseline (speedup 1.0000x reference)
"""AliNet graph-attention layer on 8 Trainium2 NeuronCores (v2).

Pipeline (3 SPMD launches; host does sharding glue + spill edges):
  L1: per-core BN partial sums over its node slice  -> host combines stats
  L2: per-core node phase: xn = BN(x), mapped = xn@K, s1/s2 = tanh(rowdot)
  host: packs table[N, 256] bf16 rows: cols 0:128 mapped, 128 s2, 129 ones;
        buckets edges into fixed 512-slot (window, chunk) cells (overflow
        edges spill to host numpy), builds gather idx + srel streams
  L3: per-core edge phase over its src-owned edges:
        dma_gather rows by dst (row-rate-bound: bytes are free),
        one-hot by srel on DVE, per-edge s1 via replicate+reduce,
        w = exp(leaky_relu(s1+s2)) (Scalar exp), scale one-hot by w,
        ONE matmul per tile accumulates num (cols 0:128) AND den (col 129)
        in PSUM; outputs raw [num|den] per node
  host: adds spill contributions, out = relu(num / max(den, 1e-16))
"""

import math
import numpy as np
import ml_dtypes

import concourse.bass as bass
import concourse.bacc as bacc
import concourse.tile as tile
import concourse.mybir as mybir
import concourse.bass_utils as bass_utils

F32 = mybir.dt.float32
BF16 = mybir.dt.bfloat16
I16 = mybir.dt.int16
I32 = mybir.dt.int32
AF = mybir.ActivationFunctionType
OP = mybir.AluOpType

BN_EPS = 1e-5
P = 128

RUN_MODE = "hw"  # "hw" or "sim"


class Cfg:
    def __init__(self, N=100000, D=128, NC=8, CHUNK=25000, QUOTA=512, GW=4):
        self.N, self.D, self.NC = N, D, NC
        assert N % NC == 0
        self.NS = N // NC                    # nodes per core
        self.NW = math.ceil(self.NS / P)     # src windows per core
        self.CHUNK = CHUNK                   # dst chunk (int16 idx range)
        self.NCH = math.ceil(N / CHUNK)      # dst chunks
        self.QUOTA = QUOTA                   # edge slots per (window, chunk)
        self.CT = QUOTA // P                 # tiles per cell
        self.GW = GW                         # windows per gather group
        self.groups = [
            list(range(i, min(i + GW, self.NW)))
            for i in range(0, self.NW, GW)
        ]
        self.TT = self.NW * self.NCH * self.CT   # total tiles per core
        self.ROW = 256                       # table row elems (bf16)


CFG = Cfg()


def _mk_nc(num_devices):
    return bacc.Bacc(
        "TRN2",
        target_bir_lowering=False,
        debug=False,
        enable_asserts=True,
        num_devices=num_devices,
    )


# ---------------------------------------------------------------- L1: stats
def build_l1(cfg):
    nc = _mk_nc(cfg.NC)
    x = nc.dram_tensor("x_slice", [cfg.NS, cfg.D], F32, kind="ExternalInput")
    stats = nc.dram_tensor("stats", [1, 2 * cfg.D], F32, kind="ExternalOutput")
    ntiles = math.ceil(cfg.NS / P)
    with tile.TileContext(nc) as tc:
        with (
            tc.tile_pool(name="sb", bufs=4) as sb,
            tc.tile_pool(name="cst", bufs=1) as cst,
            tc.tile_pool(name="ps", bufs=1, space="PSUM") as ps,
        ):
            ones = cst.tile([P, 1], F32)
            nc.gpsimd.memset(ones[:], 1.0)
            # ones is the (tiny) weights operand; x streams as the moving
            # operand so the PE never reloads a 128x128 f32 weight tile
            acc0 = ps.tile([1, cfg.D], F32, tag="a0")
            acc1 = ps.tile([1, cfg.D], F32, tag="a1")
            for t in range(ntiles):
                r0 = t * P
                rows = min(P, cfg.NS - r0)
                xt = sb.tile([P, cfg.D], F32, tag="xt")
                nc.sync.dma_start(xt[:rows, :], x[r0 : r0 + rows, :])
                xsq = sb.tile([P, cfg.D], F32, tag="xsq")
                nc.vector.tensor_tensor(
                    out=xsq[:rows, :], in0=xt[:rows, :], in1=xt[:rows, :], op=OP.mult
                )
                nc.tensor.matmul(
                    acc0[:, :], ones[:rows, :], xt[:rows, :],
                    start=(t == 0), stop=(t == ntiles - 1),
                )
                nc.tensor.matmul(
                    acc1[:, :], ones[:rows, :], xsq[:rows, :],
                    start=(t == 0), stop=(t == ntiles - 1),
                )
            out_sb = cst.tile([1, 2 * cfg.D], F32, tag="o")
            nc.vector.tensor_copy(out_sb[:, 0 : cfg.D], acc0[:])
            nc.vector.tensor_copy(out_sb[:, cfg.D : 2 * cfg.D], acc1[:])
            nc.sync.dma_start(stats[:], out_sb[:])
    nc.compile()
    return nc


# ------------------------------------------------------------ L2: node phase
def build_l2(cfg):
    nc = _mk_nc(cfg.NC)
    D, NS, NW = cfg.D, cfg.NS, cfg.NW
    xT = nc.dram_tensor("xT_slice", [D, NS], F32, kind="ExternalInput")
    scale = nc.dram_tensor("scale", [D, 1], F32, kind="ExternalInput")
    shift = nc.dram_tensor("shift", [D, 1], F32, kind="ExternalInput")
    k0 = nc.dram_tensor("k0", [D, D], F32, kind="ExternalInput")
    k1 = nc.dram_tensor("k1", [D, D], F32, kind="ExternalInput")
    k2 = nc.dram_tensor("k2", [D, D], F32, kind="ExternalInput")
    mappedT = nc.dram_tensor("mappedT", [D, NS], BF16, kind="ExternalOutput")
    s1o = nc.dram_tensor("s1o", [P, NW], F32, kind="ExternalOutput")
    s2o = nc.dram_tensor("s2o", [P, NW], F32, kind="ExternalOutput")

    with tile.TileContext(nc) as tc:
        with (
            tc.tile_pool(name="cst", bufs=1) as cst,
            tc.tile_pool(name="sb", bufs=4) as sb,
            tc.tile_pool(name="ps", bufs=6, space="PSUM") as ps,
            tc.tile_pool(name="ps1", bufs=2, space="PSUM") as ps1,
        ):
            kf = cst.tile([D, 3 * D], F32, tag="kf")
            nc.sync.dma_start(kf[:, 0:D], k0[:])
            nc.sync.dma_start(kf[:, D : 2 * D], k1[:])
            nc.sync.dma_start(kf[:, 2 * D : 3 * D], k2[:])
            kb = cst.tile([D, 3 * D], BF16, tag="kb")
            nc.vector.tensor_copy(out=kb[:], in_=kf[:])
            ksb = kb[:, 0:D]
            k1sb = kb[:, D : 2 * D]
            k2sb = kb[:, 2 * D : 3 * D]
            ssb = cst.tile([D, 1], F32, tag="sc")
            bsb = cst.tile([D, 1], F32, tag="sh")
            ones = cst.tile([D, 1], BF16, tag="on")
            s1sb = cst.tile([P, NW], F32, tag="s1")
            s2sb = cst.tile([P, NW], F32, tag="s2")
            nc.sync.dma_start(ssb[:], scale[:])
            nc.sync.dma_start(bsb[:], shift[:])
            nc.gpsimd.memset(ones[:], 1.0)
            nc.gpsimd.memset(s1sb[:], 0.0)
            nc.gpsimd.memset(s2sb[:], 0.0)

            for t in range(NW):
                c0 = t * P
                cols = min(P, NS - c0)
                xt = sb.tile([D, P], F32, tag="xt")
                nc.sync.dma_start(xt[:, :cols], xT[:, c0 : c0 + cols])
                xn = sb.tile([D, P], BF16, tag="xn")
                nc.scalar.activation(
                    out=xn[:, :cols], in_=xt[:, :cols], func=AF.Identity,
                    bias=bsb[:, 0:1], scale=ssb[:, 0:1],
                )
                mps = ps.tile([D, P], F32, tag="mm")
                nc.tensor.matmul(mps[:, :cols], ksb, xn[:, :cols],
                                 start=True, stop=True)
                msb = sb.tile([D, P], BF16, tag="ms")
                nc.vector.tensor_copy(out=msb[:, :cols], in_=mps[:, :cols])
                nc.sync.dma_start(mappedT[:, c0 : c0 + cols], msb[:, :cols])
                for (kw, ssl) in ((k1sb, s1sb), (k2sb, s2sb)):
                    yps = ps.tile([D, P], F32, tag="mm")
                    nc.tensor.matmul(yps[:, :cols], kw, xn[:, :cols],
                                     start=True, stop=True)
                    z = sb.tile([D, P], BF16, tag="z")
                    nc.vector.tensor_tensor(
                        out=z[:, :cols], in0=yps[:, :cols], in1=xn[:, :cols],
                        op=OP.mult,
                    )
                    sps = ps1.tile([P, 1], F32, tag="s")
                    nc.tensor.matmul(sps[:cols, :], z[:, :cols], ones[:],
                                     start=True, stop=True)
                    nc.scalar.activation(
                        out=ssl[:cols, t : t + 1], in_=sps[:cols, :], func=AF.Tanh
                    )
            nc.sync.dma_start(s1o[:], s1sb[:])
            nc.sync.dma_start(s2o[:], s2sb[:])
    nc.compile()
    return nc


# ------------------------------------------------------------ L3: edge phase
def build_l3(cfg):
    nc = _mk_nc(cfg.NC)
    NS, NW, NCH, CT, ROW = cfg.NS, cfg.NW, cfg.NCH, cfg.CT, cfg.ROW
    TT = cfg.TT
    IC = TT * P // 16  # idx cols (int16, 16-wrap)

    table = nc.dram_tensor("table", [cfg.N, ROW], BF16, kind="ExternalInput")
    idxs_d = nc.dram_tensor("idxs", [P, IC], I16, kind="ExternalInput")
    srel_d = nc.dram_tensor("srel", [P, TT], F32, kind="ExternalInput")
    s1r_d = nc.dram_tensor("s1rows", [NW, P], F32, kind="ExternalInput")
    out_d = nc.dram_tensor("out", [NS, 130], F32, kind="ExternalOutput")

    iota_np = np.broadcast_to(
        np.arange(P, dtype=np.float32), (P, P)
    ).astype(ml_dtypes.bfloat16)
    iota_dram = nc.inline_tensor(np.ascontiguousarray(iota_np), name="iota_c")
    ones1p_dram = nc.inline_tensor(np.ones((1, P), np.float32), name="ones1p_c")

    with tile.TileContext(nc) as tc:
        with (
            tc.tile_pool(name="cst", bufs=1) as cst,
            tc.tile_pool(name="gb", bufs=2) as gbp,
            tc.tile_pool(name="s01", bufs=2) as s01p,
            tc.tile_pool(name="wb", bufs=2) as wbp,
            tc.tile_pool(name="jk", bufs=4) as jkp,
            tc.tile_pool(name="sw", bufs=4) as swp,
            tc.tile_pool(name="sm", bufs=4) as smp,
            tc.tile_pool(name="rp", bufs=2) as rpp,
            tc.tile_pool(name="acc", bufs=1, space="PSUM") as accp,
            tc.tile_pool(name="rps", bufs=2, space="PSUM") as rpsp,
        ):
            iota_bf = cst.tile([P, P], BF16, tag="iota")
            nc.sync.dma_start(iota_bf[:], iota_dram.ap())
            ones1p = cst.tile([1, P], F32, tag="ones1p")
            nc.sync.dma_start(ones1p[:], ones1p_dram.ap())
            idx_sb = cst.tile([P, IC], I16, tag="idx")
            nc.sync.dma_start(idx_sb[:], idxs_d[:])
            srel_sb = cst.tile([P, TT], F32, tag="srel")
            nc.sync.dma_start(srel_sb[:], srel_d[:])

            gt0 = 0     # global tile base of current group
            ic0 = 0     # global idx col base
            for g in cfg.groups:
                L = len(g)
                ntile = L * NCH * CT           # tiles in this group
                gbuf = gbp.tile([P, ntile, ROW], BF16, tag="gbuf")
                for c in range(NCH):
                    n_idx = L * cfg.QUOTA
                    nc.gpsimd.dma_gather(
                        out_ap=gbuf[:, c * L * CT : (c + 1) * L * CT, :],
                        in_ap=table[
                            c * cfg.CHUNK : min((c + 1) * cfg.CHUNK, cfg.N), :
                        ],
                        idxs_ap=idx_sb[:, ic0 : ic0 + n_idx // 16],
                        num_idxs=n_idx,
                        num_idxs_reg=n_idx,
                        elem_size=ROW,
                        single_packet=False,
                    )
                    ic0 += n_idx // 16

                # replicate s1 rows of this group's windows across partitions
                repl = rpp.tile([P, L, P], BF16, tag="repl")
                for wi, w in enumerate(g):
                    s1row = smp.tile([1, P], F32, tag="s1row")
                    nc.sync.dma_start(s1row[:], s1r_d[w : w + 1, :])
                    rps = rpsp.tile([P, P], F32, tag="rps")
                    nc.tensor.matmul(rps[:], ones1p[:], s1row[:],
                                     start=True, stop=True)
                    nc.scalar.copy(out=repl[:, wi, :], in_=rps[:])

                # one-hots + per-edge s1 (tbat) per tile — no gather deps,
                # issued first so the DVE never idles behind gather waits
                s01g = s01p.tile([P, ntile, P], BF16, tag="s01")
                tbat = wbp.tile([P, ntile, 1], F32, tag="tbat")
                for t in range(ntile):
                    wi = (t % (L * CT)) // CT
                    nc.vector.tensor_scalar(
                        out=s01g[:, t, :],
                        in0=iota_bf[:],
                        scalar1=srel_sb[:, gt0 + t : gt0 + t + 1],
                        scalar2=None,
                        op0=OP.is_equal,
                    )
                    junk = jkp.tile([P, P], BF16, tag="junk")
                    nc.vector.scalar_tensor_tensor(
                        out=junk[:],
                        in0=s01g[:, t, :],
                        scalar=1.0,
                        in1=repl[:, wi, :],
                        op0=OP.mult,
                        op1=OP.mult,
                        accum_out=tbat[:, t, :],
                    )

                # per chunk-section: w-chain depends only on that section's
                # gather call, so compute overlaps the remaining calls
                accs = {}
                for wi, w in enumerate(g):
                    accs[wi] = accp.tile([P, 130], F32, tag=f"acc{wi}",
                                         name=f"acc{wi}")
                sec = L * CT
                for c in range(NCH):
                    t0 = c * sec
                    s2f = wbp.tile([P, sec, 1], F32, tag="s2f")
                    nc.scalar.copy(out=s2f[:],
                                   in_=gbuf[:, t0 : t0 + sec, 128:129])
                    wq = wbp.tile([P, sec, 1], F32, tag="wq")
                    nc.vector.tensor_tensor(
                        out=wq[:], in0=tbat[:, t0 : t0 + sec, :], in1=s2f[:],
                        op=OP.add)
                    nc.vector.scalar_tensor_tensor(
                        out=wq[:], in0=wq[:], scalar=0.01, in1=wq[:],
                        op0=OP.mult, op1=OP.max,
                    )
                    wf = wbp.tile([P, sec, 1], F32, tag="wf")
                    nc.scalar.activation(out=wf[:], in_=wq[:], func=AF.Exp)

                    # scale one-hots by w; ONE matmul per tile: num cols
                    # 0:128, (dead col 128), den col 129 (table ones column)
                    for ts in range(sec):
                        t = t0 + ts
                        wi = ts // CT
                        k = ts % CT
                        s01w = swp.tile([P, P], BF16, tag="s01w")
                        nc.scalar.activation(
                            out=s01w[:], in_=s01g[:, t, :], func=AF.Identity,
                            scale=wf[:, ts, :],
                        )
                        nc.tensor.matmul(
                            accs[wi][:, 0:130], s01w[:], gbuf[:, t, 0:130],
                            start=(c == 0 and k == 0),
                            stop=(c == NCH - 1 and k == CT - 1),
                        )

                # finalize windows: write raw [num | s2dead | den]
                for wi, w in enumerate(g):
                    rows = min(P, NS - w * P)
                    osb = smp.tile([P, 130], F32, tag="osb")
                    nc.scalar.copy(out=osb[:rows, :], in_=accs[wi][:rows, :])
                    nc.sync.dma_start(out_d[w * P : w * P + rows, :],
                                      osb[:rows, :])
                gt0 += ntile
    nc.compile()
    return nc


# ------------------------------------------------------------ host planning
def plan_edges(edge_index, cfg):
    """Bucket edges into fixed QUOTA-slot (window, chunk) cells per core.

    Returns per-core streams {idxs, srel} and the spilled edge arrays."""
    src = np.asarray(edge_index[0], dtype=np.int64)
    dst = np.asarray(edge_index[1], dtype=np.int64)
    NC, NS, NW, NCH, Q = cfg.NC, cfg.NS, cfg.NW, cfg.NCH, cfg.QUOTA
    CH, CT = cfg.CHUNK, cfg.CT
    owner = src // NS
    w = (src % NS) // P
    srel_v = (src % NS) % P
    ch = dst // CH
    key = (owner * NW + w) * NCH + ch
    order = np.argsort(key, kind="stable")
    key_s = key[order]
    bounds = np.searchsorted(key_s, np.arange(NC * NW * NCH + 1))

    # slot order within a core: for g in groups: for c: for w in g: Q slots
    cell_slot = np.empty((NW, NCH), np.int64)
    pos = 0
    for g in cfg.groups:
        for c in range(NCH):
            for ww in g:
                cell_slot[ww, c] = pos
                pos += Q
    nslot = pos
    assert nslot == cfg.TT * P

    streams = []
    spill_parts = []
    for core in range(NC):
        idx_arr = np.zeros(nslot, np.int16)
        srel_arr = np.full(nslot, 200.0, np.float32)
        for ww in range(NW):
            for c in range(NCH):
                b = (core * NW + ww) * NCH + c
                lo, hi = bounds[b], bounds[b + 1]
                take = min(Q, hi - lo)
                sel = order[lo : lo + take]
                base = cell_slot[ww, c]
                idx_arr[base : base + take] = (dst[sel] - c * CH).astype(
                    np.int16)
                srel_arr[base : base + take] = srel_v[sel].astype(np.float32)
                if hi - lo > Q:
                    spill_parts.append(order[lo + Q : hi])
        # wrap idx per gather call (call = L*Q consecutive slots)
        blocks = []
        s0 = 0
        for g in cfg.groups:
            L = len(g)
            for c in range(NCH):
                n = L * Q
                blk = idx_arr[s0 : s0 + n]
                blocks.append(np.tile(blk.reshape(-1, 16).T, (8, 1)))
                s0 += n
        idxs = np.ascontiguousarray(np.concatenate(blocks, axis=1))
        srel_T = np.ascontiguousarray(srel_arr.reshape(-1, P).T)
        streams.append({"idxs": idxs, "srel": srel_T})
    spill = (np.concatenate(spill_parts) if spill_parts
             else np.zeros(0, np.int64))
    return streams, src[spill], dst[spill]


# ------------------------------------------------------------ orchestration
def _run(nc, in_maps, cfg, **kw):
    if RUN_MODE == "sim":
        from concourse.bass_interp import MultiCoreSim

        sim = MultiCoreSim(nc, num_cores=cfg.NC, trace=False)
        for ci, core in enumerate(sim.cores.values()):
            for name, arr in in_maps[ci].items():
                core.tensor(name)[:] = arr
        sim.simulate(check_with_hw=False)
        out_names = []
        for alloc in nc.m.functions[0].allocations:
            if not isinstance(alloc, mybir.MemoryLocationSet):
                continue
            if alloc.kind == "ExternalOutput":
                out_names.append(alloc.memorylocations[0].name)
        results = [
            {n: np.array(core.tensor(n)) for n in out_names}
            for core in sim.cores.values()
        ]

        class R:
            pass

        r = R()
        r.results = results
        r.exec_time_ns = None
        return r
    return bass_utils.run_bass_kernel_spmd(
        nc, in_maps, core_ids=list(range(cfg.NC)), **kw
    )


def kernel(x, edge_index, kernel, kernel1, kernel2, gamma, beta, _cfg=None,
           _trace=False):
    cfg = _cfg or CFG
    x = np.asarray(x, np.float32)
    k0 = np.asarray(kernel, np.float32)
    k1 = np.asarray(kernel1, np.float32)
    k2 = np.asarray(kernel2, np.float32)
    gamma = np.asarray(gamma, np.float32)
    beta = np.asarray(beta, np.float32)
    NC, NS, D = cfg.NC, cfg.NS, cfg.D

    import time as _t
    _lap_t = [_t.time()]

    def _lap(msg):
        now = _t.time()
        print(f"[kernel] {msg}: +{now - _lap_t[0]:.1f}s", flush=True)
        _lap_t[0] = now

    # ---- L1
    nc1 = build_l1(cfg)
    _lap("build_l1")
    in1 = [{"x_slice": np.ascontiguousarray(x[c * NS : (c + 1) * NS])}
           for c in range(NC)]
    r1 = _run(nc1, in1, cfg, trace=_trace)
    _lap("run_l1")
    parts = np.stack([r1.results[c]["stats"][0] for c in range(NC)])
    tot = parts.sum(axis=0).astype(np.float64)
    mean = tot[0 : D] / cfg.N
    var = tot[D : 2 * D] / cfg.N - mean * mean
    rstd = gamma.astype(np.float64) / np.sqrt(var + BN_EPS)
    scale = rstd.astype(np.float32)
    shift = (beta.astype(np.float64) - mean * rstd).astype(np.float32)

    # ---- L2
    nc2 = build_l2(cfg)
    _lap("build_l2")
    in2 = []
    for c in range(NC):
        in2.append({
            "xT_slice": np.ascontiguousarray(x[c * NS : (c + 1) * NS].T),
            "scale": np.ascontiguousarray(scale.reshape(D, 1)),
            "shift": np.ascontiguousarray(shift.reshape(D, 1)),
            "k0": k0, "k1": k1, "k2": k2,
        })
    r2 = _run(nc2, in2, cfg, trace=_trace)
    _lap("run_l2")
    mapped = np.concatenate(
        [np.asarray(r2.results[c]["mappedT"]).astype(np.float32).T
         for c in range(NC)], axis=0
    )
    s1 = np.concatenate(
        [np.asarray(r2.results[c]["s1o"]).T.reshape(-1)[:NS] for c in range(NC)]
    )
    s2 = np.concatenate(
        [np.asarray(r2.results[c]["s2o"]).T.reshape(-1)[:NS] for c in range(NC)]
    )

    # ---- host glue: table + edge streams
    tbl = np.zeros((cfg.N, cfg.ROW), ml_dtypes.bfloat16)
    tbl[:, 0:128] = mapped.astype(ml_dtypes.bfloat16)
    tbl[:, 128] = s2.astype(ml_dtypes.bfloat16)
    tbl[:, 129] = 1.0
    streams, sp_src, sp_dst = plan_edges(edge_index, cfg)
    _lap(f"host_glue (spill={len(sp_src)})")

    # ---- L3
    nc3 = build_l3(cfg)
    _lap("build_l3")
    in3 = []
    for c in range(NC):
        s1pad = np.zeros(cfg.NW * P, np.float32)
        s1pad[:NS] = s1[c * NS : (c + 1) * NS]
        in3.append({
            "table": tbl,
            "idxs": streams[c]["idxs"],
            "srel": streams[c]["srel"],
            "s1rows": np.ascontiguousarray(s1pad.reshape(cfg.NW, P)),
        })
    r3 = _run(nc3, in3, cfg, trace=_trace)
    _lap("run_l3")
    raw = np.concatenate(
        [np.asarray(r3.results[c]["out"]) for c in range(NC)], axis=0
    )
    num = raw[:, 0:128].astype(np.float64)
    den = raw[:, 129].astype(np.float64)

    # ---- spill edges on host
    if len(sp_src):
        e = s1[sp_src] + s2[sp_dst]
        el = np.where(e > 0, e, 0.01 * e)
        wsp = np.exp(el).astype(np.float64)
        mb = tbl[:, 0:128].astype(np.float32).astype(np.float64)
        np.add.at(num, sp_src, wsp[:, None] * mb[sp_dst])
        np.add.at(den, sp_src, wsp)

    out = np.maximum(num / np.maximum(den, 1e-16)[:, None], 0.0)
    globals()["_LAST_RESULTS"] = (r1, r2, r3)
    return out.astype(np.float32)


# revision 15
# speedup vs baseline: 1.0542x; 1.0542x over previous
"""AliNet graph-attention layer on 8 Trainium2 NeuronCores (v2).

Pipeline (3 SPMD launches; host does sharding glue + spill edges):
  L1: per-core BN partial sums over its node slice  -> host combines stats
  L2: per-core node phase: xn = BN(x), mapped = xn@K, s1/s2 = tanh(rowdot)
  host: packs table[N, 256] bf16 rows: cols 0:128 mapped, 128 s2, 129 ones;
        buckets edges into fixed 512-slot (window, chunk) cells (overflow
        edges spill to host numpy), builds gather idx + srel streams
  L3: per-core edge phase over its src-owned edges:
        dma_gather rows by dst (row-rate-bound: bytes are free),
        one-hot by srel on DVE, per-edge s1 via replicate+reduce,
        w = exp(leaky_relu(s1+s2)) (Scalar exp), scale one-hot by w,
        ONE matmul per tile accumulates num (cols 0:128) AND den (col 129)
        in PSUM; outputs raw [num|den] per node
  host: adds spill contributions, out = relu(num / max(den, 1e-16))
"""

import math
import numpy as np
import ml_dtypes

import concourse.bass as bass
import concourse.bacc as bacc
import concourse.tile as tile
import concourse.mybir as mybir
import concourse.bass_utils as bass_utils

F32 = mybir.dt.float32
BF16 = mybir.dt.bfloat16
I16 = mybir.dt.int16
I32 = mybir.dt.int32
AF = mybir.ActivationFunctionType
OP = mybir.AluOpType

BN_EPS = 1e-5
P = 128

RUN_MODE = "hw"  # "hw" or "sim"


class Cfg:
    def __init__(self, N=100000, D=128, NC=8, CHUNK=25000, QUOTA=512, GW=4):
        self.N, self.D, self.NC = N, D, NC
        assert N % NC == 0
        self.NS = N // NC                    # nodes per core
        self.NW = math.ceil(self.NS / P)     # src windows per core
        self.CHUNK = CHUNK                   # dst chunk (int16 idx range)
        self.NCH = math.ceil(N / CHUNK)      # dst chunks
        self.QUOTA = QUOTA                   # edge slots per (window, chunk)
        self.CT = QUOTA // P                 # tiles per cell
        self.GW = GW                         # windows per gather group
        self.groups = [
            list(range(i, min(i + GW, self.NW)))
            for i in range(0, self.NW, GW)
        ]
        self.TT = self.NW * self.NCH * self.CT   # total tiles per core
        self.ROW = 256                       # table row elems (bf16)


CFG = Cfg()


def _mk_nc(num_devices):
    return bacc.Bacc(
        "TRN2",
        target_bir_lowering=False,
        debug=False,
        enable_asserts=True,
        num_devices=num_devices,
    )


# ---------------------------------------------------------------- L1: stats
def build_l1(cfg):
    nc = _mk_nc(cfg.NC)
    x = nc.dram_tensor("x_slice", [cfg.NS, cfg.D], F32, kind="ExternalInput")
    stats = nc.dram_tensor("stats", [1, 2 * cfg.D], F32, kind="ExternalOutput")
    ntiles = math.ceil(cfg.NS / P)
    with tile.TileContext(nc) as tc:
        with (
            tc.tile_pool(name="sb", bufs=4) as sb,
            tc.tile_pool(name="cst", bufs=1) as cst,
            tc.tile_pool(name="ps", bufs=1, space="PSUM") as ps,
        ):
            ones = cst.tile([P, 1], F32)
            nc.gpsimd.memset(ones[:], 1.0)
            # ones is the (tiny) weights operand; x streams as the moving
            # operand so the PE never reloads a 128x128 f32 weight tile
            acc0 = ps.tile([1, cfg.D], F32, tag="a0")
            acc1 = ps.tile([1, cfg.D], F32, tag="a1")
            for t in range(ntiles):
                r0 = t * P
                rows = min(P, cfg.NS - r0)
                xt = sb.tile([P, cfg.D], F32, tag="xt")
                nc.sync.dma_start(xt[:rows, :], x[r0 : r0 + rows, :])
                xsq = sb.tile([P, cfg.D], F32, tag="xsq")
                nc.vector.tensor_tensor(
                    out=xsq[:rows, :], in0=xt[:rows, :], in1=xt[:rows, :], op=OP.mult
                )
                nc.tensor.matmul(
                    acc0[:, :], ones[:rows, :], xt[:rows, :],
                    start=(t == 0), stop=(t == ntiles - 1),
                )
                nc.tensor.matmul(
                    acc1[:, :], ones[:rows, :], xsq[:rows, :],
                    start=(t == 0), stop=(t == ntiles - 1),
                )
            out_sb = cst.tile([1, 2 * cfg.D], F32, tag="o")
            nc.vector.tensor_copy(out_sb[:, 0 : cfg.D], acc0[:])
            nc.vector.tensor_copy(out_sb[:, cfg.D : 2 * cfg.D], acc1[:])
            nc.sync.dma_start(stats[:], out_sb[:])
    nc.compile()
    return nc


# ------------------------------------------------------------ L2: node phase
def build_l2(cfg):
    nc = _mk_nc(cfg.NC)
    D, NS, NW = cfg.D, cfg.NS, cfg.NW
    xT = nc.dram_tensor("xT_slice", [D, NS], F32, kind="ExternalInput")
    scale = nc.dram_tensor("scale", [D, 1], F32, kind="ExternalInput")
    shift = nc.dram_tensor("shift", [D, 1], F32, kind="ExternalInput")
    k0 = nc.dram_tensor("k0", [D, D], F32, kind="ExternalInput")
    k1 = nc.dram_tensor("k1", [D, D], F32, kind="ExternalInput")
    k2 = nc.dram_tensor("k2", [D, D], F32, kind="ExternalInput")
    mappedT = nc.dram_tensor("mappedT", [D, NS], BF16, kind="ExternalOutput")
    s1o = nc.dram_tensor("s1o", [P, NW], F32, kind="ExternalOutput")
    s2o = nc.dram_tensor("s2o", [P, NW], F32, kind="ExternalOutput")

    with tile.TileContext(nc) as tc:
        with (
            tc.tile_pool(name="cst", bufs=1) as cst,
            tc.tile_pool(name="sb", bufs=4) as sb,
            tc.tile_pool(name="ps", bufs=6, space="PSUM") as ps,
            tc.tile_pool(name="ps1", bufs=2, space="PSUM") as ps1,
        ):
            kf = cst.tile([D, 3 * D], F32, tag="kf")
            nc.sync.dma_start(kf[:, 0:D], k0[:])
            nc.sync.dma_start(kf[:, D : 2 * D], k1[:])
            nc.sync.dma_start(kf[:, 2 * D : 3 * D], k2[:])
            kb = cst.tile([D, 3 * D], BF16, tag="kb")
            nc.vector.tensor_copy(out=kb[:], in_=kf[:])
            ksb = kb[:, 0:D]
            k1sb = kb[:, D : 2 * D]
            k2sb = kb[:, 2 * D : 3 * D]
            ssb = cst.tile([D, 1], F32, tag="sc")
            bsb = cst.tile([D, 1], F32, tag="sh")
            ones = cst.tile([D, 1], BF16, tag="on")
            s1sb = cst.tile([P, NW], F32, tag="s1")
            s2sb = cst.tile([P, NW], F32, tag="s2")
            nc.sync.dma_start(ssb[:], scale[:])
            nc.sync.dma_start(bsb[:], shift[:])
            nc.gpsimd.memset(ones[:], 1.0)
            nc.gpsimd.memset(s1sb[:], 0.0)
            nc.gpsimd.memset(s2sb[:], 0.0)

            for t in range(NW):
                c0 = t * P
                cols = min(P, NS - c0)
                xt = sb.tile([D, P], F32, tag="xt")
                nc.sync.dma_start(xt[:, :cols], xT[:, c0 : c0 + cols])
                xn = sb.tile([D, P], BF16, tag="xn")
                nc.scalar.activation(
                    out=xn[:, :cols], in_=xt[:, :cols], func=AF.Identity,
                    bias=bsb[:, 0:1], scale=ssb[:, 0:1],
                )
                mps = ps.tile([D, P], F32, tag="mm")
                nc.tensor.matmul(mps[:, :cols], ksb, xn[:, :cols],
                                 start=True, stop=True)
                msb = sb.tile([D, P], BF16, tag="ms")
                nc.vector.tensor_copy(out=msb[:, :cols], in_=mps[:, :cols])
                nc.sync.dma_start(mappedT[:, c0 : c0 + cols], msb[:, :cols])
                for (kw, ssl) in ((k1sb, s1sb), (k2sb, s2sb)):
                    yps = ps.tile([D, P], F32, tag="mm")
                    nc.tensor.matmul(yps[:, :cols], kw, xn[:, :cols],
                                     start=True, stop=True)
                    z = sb.tile([D, P], BF16, tag="z")
                    nc.vector.tensor_tensor(
                        out=z[:, :cols], in0=yps[:, :cols], in1=xn[:, :cols],
                        op=OP.mult,
                    )
                    sps = ps1.tile([P, 1], F32, tag="s")
                    nc.tensor.matmul(sps[:cols, :], z[:, :cols], ones[:],
                                     start=True, stop=True)
                    nc.scalar.activation(
                        out=ssl[:cols, t : t + 1], in_=sps[:cols, :], func=AF.Tanh
                    )
            nc.sync.dma_start(s1o[:], s1sb[:])
            nc.sync.dma_start(s2o[:], s2sb[:])
    nc.compile()
    return nc


# ------------------------------------------------------------ L3: edge phase
def build_l3(cfg):
    nc = _mk_nc(cfg.NC)
    NS, NW, NCH, CT, ROW = cfg.NS, cfg.NW, cfg.NCH, cfg.CT, cfg.ROW
    TT = cfg.TT
    IC = TT * P // 16  # idx cols (int16, 16-wrap)

    table = nc.dram_tensor("table", [cfg.N, ROW], BF16, kind="ExternalInput")
    idxs_d = nc.dram_tensor("idxs", [P, IC], I16, kind="ExternalInput")
    srel_d = nc.dram_tensor("srel", [P, TT], F32, kind="ExternalInput")
    s1r_d = nc.dram_tensor("s1rows", [NW, P], F32, kind="ExternalInput")
    out_d = nc.dram_tensor("out", [NS, 130], F32, kind="ExternalOutput")

    iota_np = np.broadcast_to(
        np.arange(P, dtype=np.float32), (P, P)
    ).astype(ml_dtypes.bfloat16)
    iota_dram = nc.inline_tensor(np.ascontiguousarray(iota_np), name="iota_c")
    ones1p_dram = nc.inline_tensor(np.ones((1, P), np.float32), name="ones1p_c")

    with tile.TileContext(nc) as tc:
        with (
            tc.tile_pool(name="cst", bufs=1) as cst,
            tc.tile_pool(name="gb", bufs=3) as gbp,
            tc.tile_pool(name="s01", bufs=2) as s01p,
            tc.tile_pool(name="wb", bufs=2) as wbp,
            tc.tile_pool(name="jk", bufs=4) as jkp,
            tc.tile_pool(name="sw", bufs=4) as swp,
            tc.tile_pool(name="sm", bufs=4) as smp,
            tc.tile_pool(name="rp", bufs=2) as rpp,
            tc.tile_pool(name="acc", bufs=1, space="PSUM") as accp,
            tc.tile_pool(name="rps", bufs=2, space="PSUM") as rpsp,
        ):
            iota_bf = cst.tile([P, P], BF16, tag="iota")
            nc.sync.dma_start(iota_bf[:], iota_dram.ap())
            ones1p = cst.tile([1, P], F32, tag="ones1p")
            nc.sync.dma_start(ones1p[:], ones1p_dram.ap())
            idx_sb = cst.tile([P, IC], I16, tag="idx")
            nc.sync.dma_start(idx_sb[:], idxs_d[:])
            srel_sb = cst.tile([P, TT], F32, tag="srel")
            nc.sync.dma_start(srel_sb[:], srel_d[:])

            gt0 = 0     # global tile base of current group
            ic0 = 0     # global idx col base
            for g in cfg.groups:
                L = len(g)
                ntile = L * NCH * CT           # tiles in this group
                gbuf = gbp.tile([P, ntile, ROW], BF16, tag="gbuf")
                for c in range(NCH):
                    n_idx = L * cfg.QUOTA
                    nc.gpsimd.dma_gather(
                        out_ap=gbuf[:, c * L * CT : (c + 1) * L * CT, :],
                        in_ap=table[
                            c * cfg.CHUNK : min((c + 1) * cfg.CHUNK, cfg.N), :
                        ],
                        idxs_ap=idx_sb[:, ic0 : ic0 + n_idx // 16],
                        num_idxs=n_idx,
                        num_idxs_reg=n_idx,
                        elem_size=ROW,
                        single_packet=False,
                    )
                    ic0 += n_idx // 16

                # replicate s1 rows of this group's windows across partitions
                repl = rpp.tile([P, L, P], BF16, tag="repl")
                for wi, w in enumerate(g):
                    s1row = smp.tile([1, P], F32, tag="s1row")
                    nc.sync.dma_start(s1row[:], s1r_d[w : w + 1, :])
                    rps = rpsp.tile([P, P], F32, tag="rps")
                    nc.tensor.matmul(rps[:], ones1p[:], s1row[:],
                                     start=True, stop=True)
                    nc.scalar.copy(out=repl[:, wi, :], in_=rps[:])

                # one-hots + per-edge s1 (tbat) per tile — no gather deps,
                # issued first so the DVE never idles behind gather waits
                s01g = s01p.tile([P, ntile, P], BF16, tag="s01")
                tbat = wbp.tile([P, ntile, 1], F32, tag="tbat")
                for t in range(ntile):
                    wi = (t % (L * CT)) // CT
                    nc.vector.tensor_scalar(
                        out=s01g[:, t, :],
                        in0=iota_bf[:],
                        scalar1=srel_sb[:, gt0 + t : gt0 + t + 1],
                        scalar2=None,
                        op0=OP.is_equal,
                    )
                    junk = jkp.tile([P, P], BF16, tag="junk")
                    nc.vector.scalar_tensor_tensor(
                        out=junk[:],
                        in0=s01g[:, t, :],
                        scalar=1.0,
                        in1=repl[:, wi, :],
                        op0=OP.mult,
                        op1=OP.mult,
                        accum_out=tbat[:, t, :],
                    )

                # per chunk-section: w-chain depends only on that section's
                # gather call, so compute overlaps the remaining calls
                accs = {}
                for wi, w in enumerate(g):
                    accs[wi] = accp.tile([P, 130], F32, tag=f"acc{wi}",
                                         name=f"acc{wi}")
                sec = L * CT
                for c in range(NCH):
                    t0 = c * sec
                    s2f = wbp.tile([P, sec, 1], F32, tag="s2f")
                    nc.scalar.copy(out=s2f[:],
                                   in_=gbuf[:, t0 : t0 + sec, 128:129])
                    wq = wbp.tile([P, sec, 1], F32, tag="wq")
                    nc.vector.tensor_tensor(
                        out=wq[:], in0=tbat[:, t0 : t0 + sec, :], in1=s2f[:],
                        op=OP.add)
                    nc.vector.scalar_tensor_tensor(
                        out=wq[:], in0=wq[:], scalar=0.01, in1=wq[:],
                        op0=OP.mult, op1=OP.max,
                    )
                    wf = wbp.tile([P, sec, 1], F32, tag="wf")
                    nc.scalar.activation(out=wf[:], in_=wq[:], func=AF.Exp)

                    # scale one-hots by w; ONE matmul per tile: num cols
                    # 0:128, (dead col 128), den col 129 (table ones column)
                    for ts in range(sec):
                        t = t0 + ts
                        wi = ts // CT
                        k = ts % CT
                        s01w = swp.tile([P, P], BF16, tag="s01w")
                        nc.scalar.activation(
                            out=s01w[:], in_=s01g[:, t, :], func=AF.Identity,
                            scale=wf[:, ts, :],
                        )
                        nc.tensor.matmul(
                            accs[wi][:, 0:130], s01w[:], gbuf[:, t, 0:130],
                            start=(c == 0 and k == 0),
                            stop=(c == NCH - 1 and k == CT - 1),
                        )

                # finalize windows: write raw [num | s2dead | den]
                for wi, w in enumerate(g):
                    rows = min(P, NS - w * P)
                    osb = smp.tile([P, 130], F32, tag="osb")
                    nc.scalar.copy(out=osb[:rows, :], in_=accs[wi][:rows, :])
                    nc.sync.dma_start(out_d[w * P : w * P + rows, :],
                                      osb[:rows, :])
                gt0 += ntile
    nc.compile()
    return nc


# ------------------------------------------------------------ host planning
def plan_edges(edge_index, cfg):
    """Bucket edges into fixed QUOTA-slot (window, chunk) cells per core.

    Returns per-core streams {idxs, srel} and the spilled edge arrays."""
    src = np.asarray(edge_index[0], dtype=np.int64)
    dst = np.asarray(edge_index[1], dtype=np.int64)
    NC, NS, NW, NCH, Q = cfg.NC, cfg.NS, cfg.NW, cfg.NCH, cfg.QUOTA
    CH, CT = cfg.CHUNK, cfg.CT
    owner = src // NS
    w = (src % NS) // P
    srel_v = (src % NS) % P
    ch = dst // CH
    key = (owner * NW + w) * NCH + ch
    order = np.argsort(key, kind="stable")
    key_s = key[order]
    bounds = np.searchsorted(key_s, np.arange(NC * NW * NCH + 1))

    # slot order within a core: for g in groups: for c: for w in g: Q slots
    cell_slot = np.empty((NW, NCH), np.int64)
    pos = 0
    for g in cfg.groups:
        for c in range(NCH):
            for ww in g:
                cell_slot[ww, c] = pos
                pos += Q
    nslot = pos
    assert nslot == cfg.TT * P

    streams = []
    spill_parts = []
    for core in range(NC):
        idx_arr = np.zeros(nslot, np.int16)
        srel_arr = np.full(nslot, 200.0, np.float32)
        for ww in range(NW):
            for c in range(NCH):
                b = (core * NW + ww) * NCH + c
                lo, hi = bounds[b], bounds[b + 1]
                take = min(Q, hi - lo)
                sel = order[lo : lo + take]
                base = cell_slot[ww, c]
                idx_arr[base : base + take] = (dst[sel] - c * CH).astype(
                    np.int16)
                srel_arr[base : base + take] = srel_v[sel].astype(np.float32)
                if hi - lo > Q:
                    spill_parts.append(order[lo + Q : hi])
        # wrap idx per gather call (call = L*Q consecutive slots)
        blocks = []
        s0 = 0
        for g in cfg.groups:
            L = len(g)
            for c in range(NCH):
                n = L * Q
                blk = idx_arr[s0 : s0 + n]
                blocks.append(np.tile(blk.reshape(-1, 16).T, (8, 1)))
                s0 += n
        idxs = np.ascontiguousarray(np.concatenate(blocks, axis=1))
        srel_T = np.ascontiguousarray(srel_arr.reshape(-1, P).T)
        streams.append({"idxs": idxs, "srel": srel_T})
    spill = (np.concatenate(spill_parts) if spill_parts
             else np.zeros(0, np.int64))
    return streams, src[spill], dst[spill]


# ------------------------------------------------------------ orchestration
def _run(nc, in_maps, cfg, **kw):
    if RUN_MODE == "sim":
        from concourse.bass_interp import MultiCoreSim

        sim = MultiCoreSim(nc, num_cores=cfg.NC, trace=False)
        for ci, core in enumerate(sim.cores.values()):
            for name, arr in in_maps[ci].items():
                core.tensor(name)[:] = arr
        sim.simulate(check_with_hw=False)
        out_names = []
        for alloc in nc.m.functions[0].allocations:
            if not isinstance(alloc, mybir.MemoryLocationSet):
                continue
            if alloc.kind == "ExternalOutput":
                out_names.append(alloc.memorylocations[0].name)
        results = [
            {n: np.array(core.tensor(n)) for n in out_names}
            for core in sim.cores.values()
        ]

        class R:
            pass

        r = R()
        r.results = results
        r.exec_time_ns = None
        return r
    return bass_utils.run_bass_kernel_spmd(
        nc, in_maps, core_ids=list(range(cfg.NC)), **kw
    )


def kernel(x, edge_index, kernel, kernel1, kernel2, gamma, beta, _cfg=None,
           _trace=False):
    cfg = _cfg or CFG
    x = np.asarray(x, np.float32)
    k0 = np.asarray(kernel, np.float32)
    k1 = np.asarray(kernel1, np.float32)
    k2 = np.asarray(kernel2, np.float32)
    gamma = np.asarray(gamma, np.float32)
    beta = np.asarray(beta, np.float32)
    NC, NS, D = cfg.NC, cfg.NS, cfg.D

    import time as _t
    _lap_t = [_t.time()]

    def _lap(msg):
        now = _t.time()
        print(f"[kernel] {msg}: +{now - _lap_t[0]:.1f}s", flush=True)
        _lap_t[0] = now

    # ---- L1
    nc1 = build_l1(cfg)
    _lap("build_l1")
    in1 = [{"x_slice": np.ascontiguousarray(x[c * NS : (c + 1) * NS])}
           for c in range(NC)]
    r1 = _run(nc1, in1, cfg, trace=_trace)
    _lap("run_l1")
    parts = np.stack([r1.results[c]["stats"][0] for c in range(NC)])
    tot = parts.sum(axis=0).astype(np.float64)
    mean = tot[0 : D] / cfg.N
    var = tot[D : 2 * D] / cfg.N - mean * mean
    rstd = gamma.astype(np.float64) / np.sqrt(var + BN_EPS)
    scale = rstd.astype(np.float32)
    shift = (beta.astype(np.float64) - mean * rstd).astype(np.float32)

    # ---- L2
    nc2 = build_l2(cfg)
    _lap("build_l2")
    in2 = []
    for c in range(NC):
        in2.append({
            "xT_slice": np.ascontiguousarray(x[c * NS : (c + 1) * NS].T),
            "scale": np.ascontiguousarray(scale.reshape(D, 1)),
            "shift": np.ascontiguousarray(shift.reshape(D, 1)),
            "k0": k0, "k1": k1, "k2": k2,
        })
    r2 = _run(nc2, in2, cfg, trace=_trace)
    _lap("run_l2")
    mapped = np.concatenate(
        [np.asarray(r2.results[c]["mappedT"]).astype(np.float32).T
         for c in range(NC)], axis=0
    )
    s1 = np.concatenate(
        [np.asarray(r2.results[c]["s1o"]).T.reshape(-1)[:NS] for c in range(NC)]
    )
    s2 = np.concatenate(
        [np.asarray(r2.results[c]["s2o"]).T.reshape(-1)[:NS] for c in range(NC)]
    )

    # ---- host glue: table + edge streams
    tbl = np.zeros((cfg.N, cfg.ROW), ml_dtypes.bfloat16)
    tbl[:, 0:128] = mapped.astype(ml_dtypes.bfloat16)
    tbl[:, 128] = s2.astype(ml_dtypes.bfloat16)
    tbl[:, 129] = 1.0
    streams, sp_src, sp_dst = plan_edges(edge_index, cfg)
    _lap(f"host_glue (spill={len(sp_src)})")

    # ---- L3
    nc3 = build_l3(cfg)
    _lap("build_l3")
    in3 = []
    for c in range(NC):
        s1pad = np.zeros(cfg.NW * P, np.float32)
        s1pad[:NS] = s1[c * NS : (c + 1) * NS]
        in3.append({
            "table": tbl,
            "idxs": streams[c]["idxs"],
            "srel": streams[c]["srel"],
            "s1rows": np.ascontiguousarray(s1pad.reshape(cfg.NW, P)),
        })
    r3 = _run(nc3, in3, cfg, trace=_trace)
    _lap("run_l3")
    raw = np.concatenate(
        [np.asarray(r3.results[c]["out"]) for c in range(NC)], axis=0
    )
    num = raw[:, 0:128].astype(np.float64)
    den = raw[:, 129].astype(np.float64)

    # ---- spill edges on host
    if len(sp_src):
        e = s1[sp_src] + s2[sp_dst]
        el = np.where(e > 0, e, 0.01 * e)
        wsp = np.exp(el).astype(np.float64)
        mb = tbl[:, 0:128].astype(np.float32).astype(np.float64)
        np.add.at(num, sp_src, wsp[:, None] * mb[sp_dst])
        np.add.at(den, sp_src, wsp)

    out = np.maximum(num / np.maximum(den, 1e-16)[:, None], 0.0)
    globals()["_LAST_RESULTS"] = (r1, r2, r3)
    return out.astype(np.float32)
